# revision 1
# baseline (speedup 1.0000x reference)
"""Trainium2 Bass kernel for nn_BasicClassifier (spiking conv classifier).

Sharding: pure data parallelism — batch 256 is split 32 samples per core
across 8 NeuronCores; params are replicated (tiny).

Per-core design. The T=1000 LIF scan is sequential; the serial chain of
DVE work paces everything, so the kernel minimizes per-tick instruction
overhead:

  - State ring: fp16 [128, 16*128] tiles (x3). Tick t's slice holds layer-1
    membranes (cols 0:96, feature f=g*128+p at (p, g*32+c)) and layer-2
    (cols 96:128, unit row x sample col), lagged SKEW ticks.
  - The fused LIF step  m' = (m*0.9 + c) - (m > 1)  is one custom-DVE op.
    Per BLOCK the 16 ticks are issued as just TWO instructions: a 1-slice
    "bridge" (reads the previous block tile's last slice) and one FUSED op
    covering the other 15 ticks whose in0 AP is its own out AP shifted one
    slice back — the DVE streams 128 cols/tick, and the ~128-cycle page
    distance covers the write-to-read latency of the self-overlap.
  - Conv + fc GEMMs write the drive c straight into a TICK-MAJOR PSUM tile
    (strided matmul outs: psum col = t*128 + g*32 + c), so the fused op's
    in1 is one contiguous fp32 PSUM AP. No engine copies anywhere.
  - spikes: ACT Sign over the fp16 ring -> sigma bf16 (one op per block);
    fc = single-bf16 halved-weight GEMM (3 matmuls) into the C tile one
    block ahead (SKEW=16); conv inputs are bf16 hi/lo K-stacked (exact to
    ~2^-16); conv_b and the fc bias ride ones-row GEMMs.
  - mem2 history: fp16 DMA from ring cols 96:128 to DRAM; host sums.
"""

import os
import sys

for _p in ("/opt/trn_rl_repo", "/opt/pypackages"):
    if _p not in sys.path:
        sys.path.insert(0, _p)

import numpy as np

import concourse.bacc as bacc
import concourse.mybir as mybir
import concourse.tile as tile
import concourse.dve_ops as dve_ops
from concourse.dve_spec import Spec, Src0, Src1, C0, C1, lower
from concourse.dve_uop import DveOpSpec
from concourse.bass_utils import run_bass_kernel_spmd

F32 = mybir.dt.float32
F16 = mybir.dt.float16
BF16 = mybir.dt.bfloat16
ALU = mybir.AluOpType
AF = mybir.ActivationFunctionType

N_CORES = 8
B_FULL, T_FULL, L_IN = 256, 1000, 30
BC = B_FULL // N_CORES      # 32 samples per core
CH, LO = 16, 24
F = CH * LO                 # 384 features
G = 3                       # feature groups of 128
J = 35                      # fc outputs
KX = L_IN + 1               # conv contraction rows (30 taps + ones row)
BLK = 16                    # ticks per block (= 4 PSUM banks of drive)
SKEW = 3 * BLK              # layer-2 lag: spikes at tick t drive m2 at t+SKEW
WIN = 160                   # ticks per x-window DMA (multiple of BLK)
BETA, THR = 0.9, 1.0

TRACE = bool(int(os.environ.get("KERNEL_TRACE", "0")))
FUSE = bool(int(os.environ.get("KERNEL_FUSE", "1")))
LAST_RESULTS = None

_LIF_OP = None


def _get_lif_op():
    """Register the fused LIF-step op in the custom-DVE table (idempotent)."""
    global _LIF_OP
    if _LIF_OP is not None:
        return _LIF_OP
    name = "LIF_STEP_ANT59"
    for op in dve_ops.OPS:
        if op.name == name:
            _LIF_OP = op
            return op
    spec = Spec(
        body=(Src0 * C0 + Src1) - (Src0 > C1),
        reference=lambda in0, in1, s0, s1, imm2: (
            (in0.astype(np.float32) * np.float32(s0)
             + in1.reshape(in0.shape))
            - (in0 > s1).astype(np.float32)
        ).astype(np.float32),
    )
    row = dve_ops._CUSTOM_DVE_ROW_BASE + len(dve_ops.OPS)
    assert row < 0x20
    dve_ops._SUB_OPCODE_FOR_NAME[name] = row
    compiled = DveOpSpec(
        name=name, opcode=row, uops=lower(spec, ver="v3"), rd1_en=True,
    )
    compiled.validate("v3")
    op = dve_ops.DveOp(name, spec, subdim=False,
                       uops_sha={"v3": compiled.sha("v3")})
    dve_ops.OPS.append(op)
    dve_ops.CUSTOM_DVE_SPECS[name] = spec
    dve_ops._COMPILE_CACHE[(name, "v3")] = compiled
    _LIF_OP = op
    return op


def _build_nc(T):
    """Build the per-core Bass program (SPMD: same program on every core)."""
    lif = _get_lif_op()
    ticks = T + SKEW                       # DVE ticks 0..T+SKEW-1
    nblk = -(-ticks // BLK)
    pad_ticks = nblk * BLK
    windows = -(-pad_ticks // WIN)
    xt_cols = windows * WIN * BC
    NB = BLK * BC                          # 512 sample-ticks per block

    nc = bacc.Bacc("TRN2", target_bir_lowering=False)

    KS = 3 * KX                            # stacked conv K: [xh; xl; xh]
    xts_d = nc.dram_tensor("xts", [KS, xt_cols], BF16, kind="ExternalInput")
    wes_d = nc.dram_tensor("wes", [KS, F], BF16, kind="ExternalInput")
    fch_d = nc.dram_tensor("fch", [128, G * J], BF16, kind="ExternalInput")
    brs_d = nc.dram_tensor("brs", [2, 128], BF16, kind="ExternalInput")
    ones_d = nc.dram_tensor("ones", [2, NB], BF16, kind="ExternalInput")
    hist_d = nc.dram_tensor("hist", [J, BC * T], F16, kind="ExternalOutput")

    with tile.TileContext(nc) as tc:
        with (
            tc.tile_pool(name="konst", bufs=1) as kp,
            tc.tile_pool(name="ring", bufs=1) as rp,
            tc.tile_pool(name="sig", bufs=2) as sgp,
            tc.tile_pool(name="xwin", bufs=3) as xp,
            tc.tile_pool(name="cdrv", bufs=2) as cbp,
            tc.tile_pool(name="cpsum", bufs=2, space="PSUM") as cp,
        ):
            # constants -> SBUF
            wes = kp.tile([KS, F], BF16, tag="wes")
            fch = kp.tile([128, G * J], BF16, tag="fch")
            brs = kp.tile([2, 128], BF16, tag="brs")
            ones = kp.tile([2, NB], BF16, tag="ones")
            negthr = kp.tile([128, 1], F32, tag="negthr")
            nc.vector.memset(negthr[:], -THR)
            for sb, dr in ((wes, wes_d), (fch, fch_d),
                           (brs, brs_d), (ones, ones_d)):
                nc.sync.dma_start(sb[:], dr[:])

            # state ring: 3 block-sized fp16 tiles of 16 slices each (the
            # third buys WAR slack so late hist DMAs don't stall the DVE)
            ringA = rp.tile([128, BLK * 128], F16, tag="ringA")
            ringB = rp.tile([128, BLK * 128], F16, tag="ringB")
            ringC = rp.tile([128, BLK * 128], F16, tag="ringC")
            rings = (ringA, ringB, ringC)
            for r in rings:
                nc.vector.memset(r[:], 0.0)
            NR = len(rings)

            xts = {}      # window idx -> xt sbuf tile
            chs = {}      # block idx -> PSUM C tile [128, 4*512] bank-major
            csts = {}     # block idx -> SBUF fp32 drive tile, tick-major

            def load_window(w):
                # chunked into 10 DMAs so latency-critical hist DMAs behind
                # them on the same queues wait ~1us, not the whole window
                if w < 0 or w >= windows or w in xts:
                    return
                ts = xp.tile([KS, WIN * BC], BF16, tag="xws")
                step = WIN * BC // 10
                for i in range(10):
                    nc.sync.dma_start(
                        ts[:, i * step:(i + 1) * step],
                        xts_d[:, w * WIN * BC + i * step:w * WIN * BC + (i + 1) * step],
                    )
                xts[w] = ts

            def ensure_psum(b):
                """Allocate block b's bank-major PSUM C tile (banks 0-2 conv,
                bank 3 bias-primed fc; psum col = g*512 + t*32 + c)."""
                if b >= nblk or b in chs:
                    return
                ch = cp.tile([128, 4 * NB], F32, tag="ch")
                chs[b] = ch
                w = (b * BLK) // WIN
                base = (b * BLK - w * WIN) * BC
                if b >= SKEW // BLK:
                    nc.tensor.matmul(
                        out=ch[:, G * NB:4 * NB],
                        lhsT=brs[:, :], rhs=ones[:, :],
                        start=True, stop=False,
                        skip_group_check=True,
                    )
                else:
                    nc.vector.memset(ch[:, G * NB:4 * NB], 0.0)
                for g in range(G):
                    nc.tensor.matmul(
                        out=ch[:, g * NB:(g + 1) * NB],
                        lhsT=wes[:, g * 128:(g + 1) * 128],
                        rhs=xts[w][:, base:base + NB],
                        start=True, stop=True,
                    )

            def conv_copies(b):
                """ACT: psum conv banks -> tick-major fp32 SBUF drive tile
                (cst col = t*128 + g*32 + c). One op per bank."""
                if b >= nblk or b in csts:
                    return
                cs = cbp.tile([128, BLK * 128], F32, tag="cs")
                csts[b] = cs
                cs4 = cs[:].rearrange("p (t g c) -> p t g c", g=4, c=BC)
                for g in range(G):
                    nc.scalar.activation(
                        out=cs4[:, :, g, :],
                        in_=chs[b][:, g * NB:(g + 1) * NB].rearrange(
                            "p (t c) -> p t c", c=BC),
                        func=AF.Copy,
                    )

            def fc_copy(b):
                """DVE: psum fc bank -> drive-tile cols 96:128 (issued after
                the block's fused op so the fc dependency never stalls)."""
                if b >= nblk:
                    return
                cs4 = csts[b][:].rearrange("p (t g c) -> p t g c", g=4, c=BC)
                nc.vector.tensor_copy(
                    out=cs4[:, :, G, :],
                    in_=chs[b][:, G * NB:4 * NB].rearrange(
                        "p (t c) -> p t c", c=BC),
                )

            def spikes_and_fc(b):
                """After block b's ticks: sigma = Sign(m1 - 1) in {-1,0,1}
                (one ACT op over the fp16 ring; halved fc weights fold the
                (sigma+1)/2), then fc (3 bf16 matmuls) into C tile b+SKEW/BLK."""
                lead = SKEW // BLK
                if b < 0 or b + lead >= nblk:
                    return
                ring4 = rings[b % NR][:].rearrange("p (t g c) -> p t g c", g=4, c=BC)
                sg = sgp.tile([128, G * NB], BF16, tag="sg")
                sg4 = sg[:].rearrange("p (g t c) -> p t g c", g=G, c=BC)
                nc.scalar.activation(
                    out=sg4, in_=ring4[:, :, 0:G, :],
                    func=AF.Sign, bias=negthr[:],
                )
                for g in range(G):
                    nc.tensor.matmul(
                        out=chs[b + lead][0:J, G * NB:4 * NB],
                        lhsT=fch[:, g * J:(g + 1) * J],
                        rhs=sg[:, g * NB:(g + 1) * NB],
                        start=False, stop=(g == G - 1),
                        skip_group_check=True,
                    )

            def hist_dma(b):
                """mem2 of DVE-tick block b = m2 ticks [16b-SKEW, ...):
                DMA straight from the fp16 ring to DRAM (host sums)."""
                t0 = b * BLK - SKEW
                if t0 < 0:
                    return
                n = min(BLK, T - t0)
                if n <= 0:
                    return
                ring3 = rings[b % NR][:].rearrange("p (t c) -> p t c", c=128)
                nc.sync.dma_start(
                    hist_d[:, t0 * BC:(t0 + n) * BC],
                    ring3[0:J, 0:n, G * BC:128],
                )

            # prologue: drive pipeline primed one block deep
            load_window(0)
            load_window(1)
            ensure_psum(0)
            ensure_psum(1)
            conv_copies(0)
            fc_copy(0)

            for b in range(nblk):
                load_window((b * BLK) // WIN + 2)
                ensure_psum(b + 2)
                spikes_and_fc(b - 1)
                conv_copies(b + 1)
                hist_dma(b - 1)

                ring = rings[b % NR]
                prev = rings[(b - 1) % NR]
                cst = csts[b]
                nt = min(BLK, ticks - b * BLK)      # ticks in this block
                # bridge: tick 0 of the block reads the previous tile's last
                nc.vector._custom_dve(
                    lif,
                    out=ring[:, 0:128],
                    in0=prev[:, (BLK - 1) * 128:BLK * 128],
                    in1=cst[:, 0:128],
                    s0=BETA, s1=THR,
                )
                if nt > 1:
                    if FUSE:
                        # one op for ticks 1..nt-1: in0 = own out shifted
                        # back one slice; the 128-col page distance covers
                        # the write->read latency of the self-overlap.
                        nc.vector._custom_dve(
                            lif,
                            out=ring[:, 128:nt * 128],
                            in0=ring[:, 0:(nt - 1) * 128],
                            in1=cst[:, 128:nt * 128],
                            s0=BETA, s1=THR,
                        )
                    else:
                        for lo in range(1, nt):
                            nc.vector._custom_dve(
                                lif,
                                out=ring[:, lo * 128:(lo + 1) * 128],
                                in0=ring[:, (lo - 1) * 128:lo * 128],
                                in1=cst[:, lo * 128:(lo + 1) * 128],
                                s0=BETA, s1=THR,
                            )
                # fc drive for the NEXT block: by now its fc matmuls are
                # done, so this DVE copy never waits
                fc_copy(b + 1)

            # epilogue: the last block's mem2 history
            hist_dma(nblk - 1)

    nc.compile()
    return nc


def _bf16_split(a):
    import ml_dtypes
    hi = a.astype(ml_dtypes.bfloat16)
    lo = (a - hi.astype(np.float32)).astype(ml_dtypes.bfloat16)
    return hi, lo


def _host_prep(x, conv_w, conv_b, fc_w, fc_b, T):
    """Build per-core input maps (numpy only)."""
    import ml_dtypes
    ticks = T + SKEW
    nblk = -(-ticks // BLK)
    windows = -(-(nblk * BLK) // WIN)
    xt_ticks = windows * WIN

    wexp = np.zeros((KX, F), np.float32)
    for c in range(CH):
        for l in range(LO):
            wexp[l:l + 7, c * LO + l] = conv_w[c, 0, :]
        wexp[L_IN, c * LO:(c + 1) * LO] = conv_b[c]
    weh, wel = _bf16_split(wexp)
    wes = np.concatenate([weh, weh, wel], axis=0)  # K-stacked [93, F]

    # spike trick: s = (sigma+1)/2 with sigma = sign(m-1) in {-1,0,1}
    # c2 = fc_w @ s + b = (fc_w/2) @ sigma + (b + fc_w.sum/2)
    half = (fc_w * 0.5).astype(np.float32)
    fcwt = np.zeros((128, G * J), np.float32)
    for g in range(G):
        fcwt[:, g * J:(g + 1) * J] = half[:, g * 128:(g + 1) * 128].T
    fch = fcwt.astype(ml_dtypes.bfloat16)
    brow = np.zeros((1, 128), np.float32)
    brow[0, :J] = fc_b + half.sum(axis=1)
    brh, brl = _bf16_split(brow)
    brs = np.concatenate([brh, brl], axis=0)       # [2, 128]

    ones = np.ones((2, BLK * BC), ml_dtypes.bfloat16)

    in_maps = []
    B = x.shape[0]
    n_cores = B // BC
    for core in range(n_cores):
        xc = x[core * BC:(core + 1) * BC]          # [BC, T, L]
        xt = np.zeros((KX, xt_ticks, BC), np.float32)
        xt[:L_IN, :T, :] = xc.transpose(2, 1, 0)
        xt[L_IN, :T, :] = 1.0
        xt = xt.reshape(KX, xt_ticks * BC)
        xth, xtl = _bf16_split(xt)
        xstk = np.concatenate([xth, xtl, xth], axis=0)  # [93, cols]
        in_maps.append({
            "xts": xstk, "wes": wes, "fch": fch,
            "brs": brs, "ones": ones,
        })
    return in_maps


def _install_trace_hook():
    """Wire up the axon NTFF profiling hook (absent from this image)."""
    import types

    if "antenv.axon_hooks" in sys.modules:
        return True
    try:
        if "/root/.axon_site" not in sys.path:
            sys.path.insert(0, "/root/.axon_site")
        from trn_agent_boot.trn_boot import _ntff_profile_via_ctypes

        hook = _ntff_profile_via_ctypes("/opt/axon/libaxon_pjrt.so")
        if hook is None:
            return False
        mod = types.ModuleType("antenv.axon_hooks")
        mod.get_axon_ntff_profile_hook = lambda: hook
        sys.modules["antenv.axon_hooks"] = mod
        import concourse.bass_utils as bu

        bu.upload_artifacts = lambda tmpdir: str(tmpdir)
        return True
    except Exception as e:  # profiling is optional
        print(f"trace hook install failed: {e}", file=sys.stderr)
        return False


def run_cores(x, conv_w, conv_b, fc_w, fc_b, T=None):
    """Run the Bass kernel on len(batch)/32 cores; returns [B, 35] output."""
    global LAST_RESULTS
    T = T if T is not None else x.shape[1]
    trace = TRACE and _install_trace_hook()
    nc = _build_nc(T)
    in_maps = _host_prep(x, conv_w, conv_b, fc_w, fc_b, T)
    res = run_bass_kernel_spmd(
        nc, in_maps, core_ids=list(range(len(in_maps))), trace=trace,
    )
    LAST_RESULTS = res
    outs = []
    for i in range(len(in_maps)):
        hv = np.asarray(res.results[i]["hist"], dtype=np.float32)
        m2 = hv.reshape(J, T, BC)                  # [J, t, sample]
        outs.append((m2.sum(axis=1) / np.float32(T)).T.astype(np.float32))
    return np.concatenate(outs, axis=0)


def kernel(x, conv_w, conv_b, fc_w, fc_b):
    return run_cores(
        np.asarray(x, np.float32), np.asarray(conv_w, np.float32),
        np.asarray(conv_b, np.float32), np.asarray(fc_w, np.float32),
        np.asarray(fc_b, np.float32),
    )

